# revision 4
# baseline (speedup 1.0000x reference)
"""Trainium2 Bass kernel for fused multi-head attention (16 heads, d=64,
b=2, n=2048, h=1024) across 8 NeuronCores.

Sharding: sequence-parallel. Flatten (b, n) -> 4096 rows; core c owns rows
[512c, 512c+512) (all 16 heads). Each core computes QKV^T for its rows,
applies rotary locally, then AllGathers K^T and V within its batch group
([[0..3],[4..7]]) so every core holds full-sequence K/V for its batch.
Attention runs in scores-transposed layout [k, q] (softmax without max
subtraction -- logits are ~N(0,1) here), the softmax denominator comes from
a ones-column appended to V, and normalization is applied to the tiny
attn_out^T [64, 512] tiles via a PE broadcast of 1/denom. The output
projection consumes attn_out^T directly as lhsT, so no on-chip transposes
are needed anywhere. All matmuls bf16 with f32 PSUM accumulation.
"""

import sys

if "/opt/trn_rl_repo" not in sys.path:
    sys.path.insert(0, "/opt/trn_rl_repo")

import numpy as np
import ml_dtypes

import concourse.bass as bass
import concourse.mybir as mybir
import concourse.tile as tile
from concourse import bacc
from concourse.bass import ts
from concourse.bass_utils import run_bass_kernel_spmd

BF16 = mybir.dt.bfloat16
F32 = mybir.dt.float32
ADD = mybir.AluOpType.add
MULT = mybir.AluOpType.mult
BYPASS = mybir.AluOpType.bypass
EXP = mybir.ActivationFunctionType.Exp

HEADS, D, H, N, B = 16, 64, 1024, 2048, 2
NC_ = 8          # cores
RPC = 512        # rows per core
PAIRS = 8        # head pairs (2 heads / 128 partitions)
KC = 16          # k chunks of 128 over n=2048
VW = HEADS * 65  # v-aug width: 64 v cols + 1 ones col per head
GROUPS = [[0, 1, 2, 3], [4, 5, 6, 7]]


def build_nc():
    nc = bacc.Bacc("TRN2", target_bir_lowering=False, debug=False, num_devices=NC_)

    xT = nc.declare_dram_parameter("xT", [H, RPC], BF16, isOutput=False)
    wqk = nc.declare_dram_parameter("wqk", [H, 2 * H], BF16, isOutput=False)
    wv = nc.declare_dram_parameter("wv", [H, VW], BF16, isOutput=False)
    wout = nc.declare_dram_parameter("wout", [H, H], BF16, isOutput=False)
    cos2 = nc.declare_dram_parameter("cos2", [128, RPC], F32, isOutput=False)
    sinm = nc.declare_dram_parameter("sinm", [D, RPC], F32, isOutput=False)
    out = nc.declare_dram_parameter("out", [RPC, H], F32, isOutput=True)

    with tile.TileContext(nc) as tc:
        with (
            tc.tile_pool(name="dram", bufs=1, space="DRAM") as dram,
            tc.tile_pool(name="sb", bufs=1) as sb,
            tc.tile_pool(name="sbw", bufs=1) as sbw,
            tc.tile_pool(name="psum", bufs=2, space="PSUM") as ps,
        ):
            kt_bounce = dram.tile([H, RPC], BF16)
            v_bounce = dram.tile([RPC, VW], BF16)
            kt_gather = dram.tile([4, H, RPC], BF16)
            v_gather = dram.tile([4, RPC, VW], BF16)

            # ---- stage in weights / activations ----
            xt_sb = sbw.tile([128, 8 * RPC], BF16)
            wqk_sb = sbw.tile([128, 8 * 2 * H], BF16)
            wv_sb = sbw.tile([128, 8 * VW], BF16)
            wout_sb = sbw.tile([128, 8 * H], BF16)
            cos2_sb = sbw.tile([128, RPC], F32)
            sinm_sb = sbw.tile([D, RPC], F32)
            ones_sb = sbw.tile([1, D], F32)
            for hk in range(8):
                nc.sync.dma_start(wqk_sb[:, ts(hk, 2 * H)], wqk[ts(hk, 128), :])
            for hk in range(8):
                nc.sync.dma_start(xt_sb[:, ts(hk, RPC)], xT[ts(hk, 128), :])
            for hk in range(8):
                nc.sync.dma_start(wv_sb[:, ts(hk, VW)], wv[ts(hk, 128), :])
            nc.sync.dma_start(cos2_sb[:, :], cos2[:, :])
            nc.sync.dma_start(sinm_sb[:, :], sinm[:, :])
            for hk in range(8):
                nc.sync.dma_start(wout_sb[:, ts(hk, H)], wout[ts(hk, 128), :])
            nc.vector.memset(ones_sb[:, :], 1.0)

            kt_rot = sb.tile([128, 8 * RPC], BF16)  # K^T rotated, local rows
            qt_rot = sb.tile([128, 8 * RPC], BF16)  # Q^T rotated, local rows
            v_loc = sb.tile([128, 4 * VW], BF16)    # V (+ones), local rows

            def project_T(dst, col0):
                """Q^T/K^T chunk dc: dst[:, dc*512:+512] = w[:, cols].T @ x^T,
                with rotary applied on the way out of PSUM."""
                for dc in range(8):
                    p = ps.tile([128, RPC], F32, tag="b")
                    for hk in range(8):
                        nc.tensor.matmul(
                            p[:, :],
                            lhsT=wqk_sb[:, hk * 2 * H + col0 + dc * 128:][:, :128],
                            rhs=xt_sb[:, ts(hk, RPC)],
                            start=(hk == 0),
                            stop=(hk == 7),
                        )
                    tmp = sb.tile([128, RPC], F32, tag="rot_a", bufs=2)
                    tmp2 = sb.tile([128, RPC], F32, tag="rot_b", bufs=2)
                    for hh in (0, 64):
                        nc.vector.tensor_tensor(
                            tmp[hh : hh + 32, :], p[hh + 32 : hh + 64, :],
                            sinm_sb[0:32, :], MULT)
                        nc.vector.tensor_tensor(
                            tmp[hh + 32 : hh + 64, :], p[hh : hh + 32, :],
                            sinm_sb[32:64, :], MULT)
                    nc.vector.tensor_tensor(tmp2[:, :], p[:, :], cos2_sb[:, :], MULT)
                    nc.vector.tensor_tensor(
                        dst[:, ts(dc, RPC)], tmp2[:, :], tmp[:, :], ADD)

            # K^T first so its AllGather starts early
            project_T(kt_rot, H)
            for dc in range(8):
                nc.sync.dma_start(kt_bounce[ts(dc, 128), :], kt_rot[:, ts(dc, RPC)])
            nc.gpsimd.collective_compute(
                "AllGather", BYPASS, replica_groups=GROUPS,
                ins=[kt_bounce.opt()], outs=[kt_gather.opt()])

            # V projection (natural orientation), ones col per head
            for rc in range(4):
                for f0, fw in ((0, 512), (512, 512), (1024, VW - 1024)):
                    p = ps.tile([128, 512], F32, tag="b")
                    for hk in range(8):
                        nc.tensor.matmul(
                            p[:, :fw],
                            lhsT=xt_sb[:, hk * RPC + rc * 128:][:, :128],
                            rhs=wv_sb[:, hk * VW + f0:][:, :fw],
                            start=(hk == 0),
                            stop=(hk == 7),
                        )
                    nc.vector.tensor_copy(
                        v_loc[:, rc * VW + f0:][:, :fw], p[:, :fw])
                ones_view = v_loc[:, rc * VW : (rc + 1) * VW].rearrange(
                    "p (h e) -> p h e", e=65)[:, :, 64:65]
                nc.vector.memset(ones_view, 1.0)
            for rc in range(4):
                nc.sync.dma_start(v_bounce[ts(rc, 128), :], v_loc[:, ts(rc, VW)])
            nc.gpsimd.collective_compute(
                "AllGather", BYPASS, replica_groups=GROUPS,
                ins=[v_bounce.opt()], outs=[v_gather.opt()])

            project_T(qt_rot, 0)

            # gathered V -> SBUF, resident across all pairs
            vt_all = sb.tile([128, KC * VW], BF16)
            for kc in range(KC):
                nc.sync.dma_start(
                    vt_all[:, ts(kc, VW)], v_gather[kc // 4, ts(kc % 4, 128), :])

            attn_sb = sb.tile([128, 8 * RPC], BF16)  # attn_out^T, normalized

            # ---- attention, per head pair ----
            for p_i in range(PAIRS):
                h0, h1 = 2 * p_i, 2 * p_i + 1
                kt_pair = sb.tile([128, N], BF16, tag="ktp", bufs=2)
                for r in range(4):
                    nc.sync.dma_start(
                        kt_pair[:, ts(r, RPC)], kt_gather[r, ts(p_i, 128), :])
                qt_p = qt_rot[:, ts(p_i, RPC)]
                av0 = ps.tile([65, RPC], F32, tag="av")
                av1 = ps.tile([65, RPC], F32, tag="av")
                exps = []
                for kc in range(KC):
                    s_ps = ps.tile([128, 2 * RPC], F32, tag="s")
                    nc.tensor.matmul(
                        s_ps[:, 0:RPC], lhsT=kt_pair[0:64, ts(kc, 128)],
                        rhs=qt_p[0:64, :], start=True, stop=True,
                        tile_position=(0, 0))
                    nc.tensor.matmul(
                        s_ps[:, RPC : 2 * RPC], lhsT=kt_pair[64:128, ts(kc, 128)],
                        rhs=qt_p[64:128, :], start=True, stop=True,
                        tile_position=(64, 0))
                    e = sb.tile([128, 2 * RPC], BF16, tag="exp", bufs=3)
                    nc.scalar.activation(e[:, :], s_ps[:, :], EXP, scale=0.125)
                    exps.append(e)
                    # software-pipelined: AV of chunk kc-1 while exp(kc) runs
                    if kc > 0:
                        _av_mm(nc, vt_all, exps[kc - 1], av0, av1, kc - 1, h0, h1)
                _av_mm(nc, vt_all, exps[KC - 1], av0, av1, KC - 1, h0, h1)

                # 1/denominator, broadcast across 64 partitions via PE
                rc0 = sb.tile([1, RPC], F32, tag="rcp", bufs=4)
                rc1 = sb.tile([1, RPC], F32, tag="rcp", bufs=4)
                nc.vector.reciprocal(rc0[:, :], av0[64:65, :])
                nc.vector.reciprocal(rc1[:, :], av1[64:65, :])
                b_ps = ps.tile([128, RPC], F32, tag="b")
                nc.tensor.matmul(b_ps[0:64, :], lhsT=ones_sb[:, :], rhs=rc0[:, :],
                                 start=True, stop=True, tile_position=(0, 0))
                nc.tensor.matmul(b_ps[64:128, :], lhsT=ones_sb[:, :], rhs=rc1[:, :],
                                 start=True, stop=True, tile_position=(0, 64))
                b_sb = sb.tile([128, RPC], F32, tag="bsb", bufs=2)
                nc.vector.tensor_copy(b_sb[:, :], b_ps[:, :])
                nc.vector.tensor_tensor(
                    attn_sb[0:64, ts(p_i, RPC)], av0[0:64, :], b_sb[0:64, :], MULT)
                nc.vector.tensor_tensor(
                    attn_sb[64:128, ts(p_i, RPC)], av1[0:64, :], b_sb[64:128, :],
                    MULT)

            # ---- output projection: out[rows, H] = attn^T.T @ w_out ----
            for rc_i in range(4):
                o_ps = ps.tile([128, H], F32, tag="s")
                for nh in range(2):
                    for hc in range(8):
                        nc.tensor.matmul(
                            o_ps[:, ts(nh, 512)],
                            lhsT=attn_sb[:, hc * RPC + rc_i * 128:][:, :128],
                            rhs=wout_sb[:, hc * H + nh * 512:][:, :512],
                            start=(hc == 0),
                            stop=(hc == 7),
                        )
                o_sb = sb.tile([128, H], F32, tag="osb", bufs=2)
                nc.vector.tensor_copy(o_sb[:, :], o_ps[:, :])
                nc.sync.dma_start(out[ts(rc_i, 128), :], o_sb[:, :])

    nc.finalize()
    return nc


def _av_mm(nc, vt_all, e, av0, av1, kc, h0, h1):
    nc.tensor.matmul(
        av0[:, :], lhsT=vt_all[:, kc * VW + 65 * h0:][:, :65],
        rhs=e[:, 0:RPC], start=(kc == 0), stop=(kc == KC - 1))
    nc.tensor.matmul(
        av1[:, :], lhsT=vt_all[:, kc * VW + 65 * h1:][:, :65],
        rhs=e[:, RPC : 2 * RPC], start=(kc == 0), stop=(kc == KC - 1))


_NC = None


def _get_nc():
    global _NC
    if _NC is None:
        _NC = build_nc()
    return _NC


def _bf16(a):
    return np.ascontiguousarray(a.astype(ml_dtypes.bfloat16))


def make_in_maps(x, rotary_emb, w_qkv, w_out):
    x = np.asarray(x, np.float32)
    rotary_emb = np.asarray(rotary_emb, np.float32)
    w_qkv = np.asarray(w_qkv, np.float32)
    w_out = np.asarray(w_out, np.float32)
    x_flat = x.reshape(B * N, H)
    wqk_bf = _bf16(w_qkv[:, : 2 * H])
    wv_aug = np.zeros((H, VW), np.float32)
    for h in range(HEADS):
        wv_aug[:, 65 * h : 65 * h + 64] = w_qkv[:, 2 * H + 64 * h : 2 * H + 64 * h + 64]
    wv_bf = _bf16(wv_aug)
    wout_bf = _bf16(w_out)
    cos = np.cos(rotary_emb)  # [N, D]
    sin = np.sin(rotary_emb)
    in_maps = []
    for c in range(NC_):
        rows = x_flat[c * RPC : (c + 1) * RPC]
        n0 = (c % 4) * RPC
        cosT = cos[n0 : n0 + RPC].T.astype(np.float32)  # [64, 512]
        sinT = sin[n0 : n0 + RPC].T.astype(np.float32)
        cos2 = np.ascontiguousarray(np.concatenate([cosT, cosT], axis=0))
        sinm = np.ascontiguousarray(
            np.concatenate([-sinT[:32], sinT[32:]], axis=0))
        in_maps.append({
            "xT": _bf16(rows.T),
            "wqk": wqk_bf,
            "wv": wv_bf,
            "wout": wout_bf,
            "cos2": cos2,
            "sinm": sinm,
        })
    return in_maps


def run(x, rotary_emb, w_qkv, w_out, trace=False, tmpdir=None):
    nc = _get_nc()
    in_maps = make_in_maps(x, rotary_emb, w_qkv, w_out)
    res = run_bass_kernel_spmd(nc, in_maps, list(range(NC_)), trace=trace,
                               tmpdir=tmpdir)
    outs = [np.asarray(res.results[c]["out"], np.float32) for c in range(NC_)]
    full = np.concatenate(outs, axis=0).reshape(B, N, H)
    return full, res


def kernel(x, rotary_emb, w_qkv, w_out):
    full, _ = run(x, rotary_emb, w_qkv, w_out)
    return full


# revision 9
# speedup vs baseline: 1.2431x; 1.2431x over previous
"""Trainium2 Bass kernel for fused multi-head attention (16 heads, d=64,
b=2, n=2048, h=1024) across 8 NeuronCores.

Sharding: sequence-parallel. Flatten (b, n) -> 4096 rows; core c owns rows
[512c, 512c+512) (all 16 heads). Each core computes QKV^T for its rows,
applies rotary locally, then AllGathers K^T and V within its batch group
([[0..3],[4..7]]) so every core holds full-sequence K/V for its batch. The
gathers are split in halves and interleaved (K0, V0, K1, V1) so attention
on head pairs 0-3 starts as soon as the first two land. Attention runs in
scores-transposed layout [k, q] (softmax without max subtraction -- logits
are ~N(0,1) here), the softmax denominator comes from a ones-column
appended to V (M=65 AV matmuls), and normalization is applied to the tiny
attn_out^T [64, 512] tiles via a PE broadcast of 1/denom. The output
projection consumes attn_out^T directly as lhsT, so no on-chip transposes
are needed anywhere. All matmuls bf16 with f32 PSUM accumulation.
"""

import sys

if "/opt/trn_rl_repo" not in sys.path:
    sys.path.insert(0, "/opt/trn_rl_repo")

import numpy as np
import ml_dtypes

import concourse.bass as bass
import concourse.mybir as mybir
import concourse.tile as tile
from concourse import bacc
from concourse.bass import ts
from concourse.bass_utils import run_bass_kernel_spmd

BF16 = mybir.dt.bfloat16
F32 = mybir.dt.float32
ADD = mybir.AluOpType.add
MULT = mybir.AluOpType.mult
BYPASS = mybir.AluOpType.bypass
EXP = mybir.ActivationFunctionType.Exp

HEADS, D, H, N, B = 16, 64, 1024, 2048, 2
NC_ = 8          # cores
RPC = 512        # rows per core
PAIRS = 8        # head pairs (2 heads / 128 partitions)
KC = 16          # k chunks of 128 over n=2048
VW = HEADS * 65  # v-aug width: 64 v cols + 1 ones col per head
HVW = VW // 2    # 520: 8 heads per half
GROUPS = [[0, 1, 2, 3], [4, 5, 6, 7]]


def build_nc():
    nc = bacc.Bacc("TRN2", target_bir_lowering=False, debug=False, num_devices=NC_)

    xT = nc.declare_dram_parameter("xT", [H, RPC], BF16, isOutput=False)
    wqk = nc.declare_dram_parameter("wqk", [H, 2 * H], BF16, isOutput=False)
    wv = nc.declare_dram_parameter("wv", [H, VW], BF16, isOutput=False)
    wout = nc.declare_dram_parameter("wout", [H, H], BF16, isOutput=False)
    cos2 = nc.declare_dram_parameter("cos2", [128, RPC], F32, isOutput=False)
    # sswp[p] = sin value read at SOURCE partition p during the shuffle:
    # p%64 < 32 -> +sin[p%64+32], else -sin[p%64-32]
    sinm = nc.declare_dram_parameter("sinm", [128, RPC], F32, isOutput=False)
    out = nc.declare_dram_parameter("out", [RPC, H], F32, isOutput=True)

    with tile.TileContext(nc) as tc:
        with (
            tc.tile_pool(name="dram", bufs=1, space="DRAM") as dram,
            tc.tile_pool(name="sb", bufs=1) as sb,
            tc.tile_pool(name="sbw", bufs=1) as sbw,
            tc.tile_pool(name="psum", bufs=2, space="PSUM") as ps,
        ):
            kt_bounce = [dram.tile([4 * 128, RPC], BF16, name=f"ktb{i}")
                         for i in range(2)]
            v_bounce = [dram.tile([RPC, HVW], BF16, name=f"vb{i}")
                        for i in range(2)]
            kt_g = [dram.tile([4, 4 * 128, RPC], BF16, name=f"ktg{i}")
                    for i in range(2)]
            v_g = [dram.tile([4, RPC, HVW], BF16, name=f"vg{i}")
                   for i in range(2)]

            # ---- stage in weights / activations ----
            # critical path first: cos/sin + x (gpsimd queue), K-half of wqk
            # (sync queue); everything else streams behind.
            xt_sb = sbw.tile([128, 8 * RPC], BF16)
            wqk_sb = sbw.tile([128, 8 * 2 * H], BF16)
            wv_sb = sbw.tile([128, 8 * VW], BF16)
            wout_sb = sbw.tile([128, 8 * H], BF16)
            cos2_sb = sbw.tile([128, RPC], F32)
            sinm_sb = sbw.tile([128, RPC], F32)
            ones_sb = sbw.tile([1, D], F32)
            nc.gpsimd.dma_start(cos2_sb[:, :], cos2[:, :])
            nc.gpsimd.dma_start(sinm_sb[:, :], sinm[:, :])
            for hk in range(8):
                nc.gpsimd.dma_start(xt_sb[:, ts(hk, RPC)], xT[ts(hk, 128), :])
            for hk in range(8):  # K columns of wqk first
                nc.sync.dma_start(
                    wqk_sb[:, hk * 2 * H + H : (hk + 1) * 2 * H],
                    wqk[ts(hk, 128), H : 2 * H])
            for hk in range(8):
                nc.sync.dma_start(wv_sb[:, ts(hk, VW)], wv[ts(hk, 128), :])
            for hk in range(8):  # Q columns
                nc.sync.dma_start(
                    wqk_sb[:, hk * 2 * H : hk * 2 * H + H],
                    wqk[ts(hk, 128), 0:H])
            for hk in range(8):
                nc.sync.dma_start(wout_sb[:, ts(hk, H)], wout[ts(hk, 128), :])
            nc.vector.memset(ones_sb[:, :], 1.0)

            kt_rot = sb.tile([128, 8 * RPC], BF16)  # K^T rotated, local rows
            qt_rot = sb.tile([128, 8 * RPC], BF16)  # Q^T rotated, local rows
            v_loc = sb.tile([128, 4 * VW], BF16)    # V (+ones), local rows

            def proj_chunk(dc, col0):
                p = ps.tile([128, RPC], F32, tag="b", name=f"pp{col0}_{dc}")
                for hk in range(8):
                    nc.tensor.matmul(
                        p[:, :],
                        lhsT=wqk_sb[:, hk * 2 * H + col0 + dc * 128:][:, :128],
                        rhs=xt_sb[:, ts(hk, RPC)],
                        start=(hk == 0),
                        stop=(hk == 7),
                    )
                return p

            def rotary(eng, p, dst_ap, tag):
                """dst = p*cos + shuffle32(p)*sinm, on engine `eng`."""
                tmp = sb.tile([128, RPC], F32, tag=tag + "a", bufs=2,
                              name=tag + "a")
                tmp2 = sb.tile([128, RPC], F32, tag=tag + "b", bufs=2,
                               name=tag + "b")
                for hh in (0, 64):
                    eng.tensor_tensor(
                        tmp[hh : hh + 32, :], p[hh + 32 : hh + 64, :],
                        sinm_sb[hh + 32 : hh + 64, :], MULT)
                    eng.tensor_tensor(
                        tmp[hh + 32 : hh + 64, :], p[hh : hh + 32, :],
                        sinm_sb[hh : hh + 32, :], MULT)
                eng.tensor_tensor(tmp2[:, :], p[:, :], cos2_sb[:, :], MULT)
                eng.tensor_tensor(dst_ap, tmp2[:, :], tmp[:, :], ADD)

            def v_proj_half(half):
                c0 = half * HVW
                for rc in range(4):
                    for fo in (0, 260):
                        p = ps.tile([128, 260], F32, tag="b",
                                    name=f"vp{half}_{rc}_{fo}")
                        for hk in range(8):
                            nc.tensor.matmul(
                                p[:, :],
                                lhsT=xt_sb[:, hk * RPC + rc * 128:][:, :128],
                                rhs=wv_sb[:, hk * VW + c0 + fo:][:, :260],
                                start=(hk == 0),
                                stop=(hk == 7),
                            )
                        nc.scalar.copy(
                            v_loc[:, rc * VW + c0 + fo:][:, :260], p[:, :])
                    ones_view = v_loc[:, rc * VW + c0 : rc * VW + c0 + HVW
                                      ].rearrange("p (h e) -> p h e", e=65
                                                  )[:, :, 64:65]
                    nc.vector.memset(ones_view, 1.0)
                nc.gpsimd.dma_start(
                    v_bounce[half].rearrange("(r p) c -> p r c", p=128),
                    v_loc.rearrange("p (r c) -> p r c", c=VW)[:, :, c0 : c0 + HVW])
                nc.gpsimd.collective_compute(
                    "AllGather", BYPASS, replica_groups=GROUPS,
                    ins=[v_bounce[half].opt()], outs=[v_g[half].opt()])

            def k_proj_half(half):
                for dc in range(4 * half, 4 * half + 4):
                    p = proj_chunk(dc, H)
                    rotary(nc.vector, p, kt_rot[:, ts(dc, RPC)], "krot")
                nc.gpsimd.dma_start(
                    kt_bounce[half].rearrange("(d p) f -> p d f", p=128),
                    kt_rot.rearrange("p (d f) -> p d f", f=RPC
                                     )[:, 4 * half : 4 * half + 4, :])
                nc.gpsimd.collective_compute(
                    "AllGather", BYPASS, replica_groups=GROUPS,
                    ins=[kt_bounce[half].opt()], outs=[kt_g[half].opt()])

            k_proj_half(0)
            v_proj_half(0)
            k_proj_half(1)
            v_proj_half(1)

            # Q projection: psum -> f32 sbuf via ACT, rotary on GpSimd
            # (off the AG critical path; DVE stays free for attention).
            for dc in range(8):
                p = proj_chunk(dc, 0)
                qf = sb.tile([128, RPC], F32, tag="qf", bufs=2, name="qf")
                nc.scalar.copy(qf[:, :], p[:, :])
                rotary(nc.gpsimd, qf, qt_rot[:, ts(dc, RPC)], "qrot")

            # gathered V -> SBUF, resident across all pairs (2 DMAs)
            vt_all = sb.tile([128, KC * VW], BF16)
            for half in range(2):
                nc.sync.dma_start(
                    vt_all.rearrange("p (k c) -> p k c", c=VW
                                     )[:, :, half * HVW : (half + 1) * HVW],
                    v_g[half].rearrange("r (j p) c -> p (r j) c", p=128))

            attn_sb = sb.tile([128, 8 * RPC], BF16)  # attn_out^T, normalized

            # ---- attention, per head pair ----
            for p_i in range(PAIRS):
                h0, h1 = 2 * p_i, 2 * p_i + 1
                kt_pair = sb.tile([128, N], BF16, tag="ktp", bufs=2,
                                  name="ktp")
                nc.sync.dma_start(
                    kt_pair.rearrange("p (r f) -> p r f", f=RPC),
                    kt_g[p_i // 4][:, ts(p_i % 4, 128), :].rearrange(
                        "r p f -> p r f"))
                qt_p = qt_rot[:, ts(p_i, RPC)]
                av0 = ps.tile([65, RPC], F32, tag="av", name="av0")
                av1 = ps.tile([65, RPC], F32, tag="av", name="av1")
                exps = []
                for kc in range(KC):
                    s_ps = ps.tile([128, 2 * RPC], F32, tag="s", name="s_ps")
                    nc.tensor.matmul(
                        s_ps[:, 0:RPC], lhsT=kt_pair[0:64, ts(kc, 128)],
                        rhs=qt_p[0:64, :], start=True, stop=True,
                        tile_position=(0, 0))
                    nc.tensor.matmul(
                        s_ps[:, RPC : 2 * RPC], lhsT=kt_pair[64:128, ts(kc, 128)],
                        rhs=qt_p[64:128, :], start=True, stop=True,
                        tile_position=(64, 0))
                    e = sb.tile([128, 2 * RPC], BF16, tag="exp", bufs=3,
                                name="e")
                    nc.scalar.activation(e[:, :], s_ps[:, :], EXP, scale=0.125)
                    exps.append(e)
                    # software-pipelined: AV of chunk kc-1 while exp(kc) runs
                    if kc > 0:
                        _av_mm(nc, vt_all, exps[kc - 1], av0, av1, kc - 1, h0, h1)
                _av_mm(nc, vt_all, exps[KC - 1], av0, av1, KC - 1, h0, h1)

                # 1/denominator, broadcast across 64 partitions via PE
                rc0 = sb.tile([1, RPC], F32, tag="rcp", bufs=4, name="rc0")
                rc1 = sb.tile([1, RPC], F32, tag="rcp", bufs=4, name="rc1")
                rd0 = sb.tile([1, RPC], F32, tag="rcd", bufs=4, name="rd0")
                rd1 = sb.tile([1, RPC], F32, tag="rcd", bufs=4, name="rd1")
                nc.vector.tensor_copy(rd0[:, :], av0[64:65, :])
                nc.vector.tensor_copy(rd1[:, :], av1[64:65, :])
                nc.vector.reciprocal_approx_fast(out=rc0[:, :], in_=rd0[:, :])
                nc.vector.reciprocal_approx_fast(out=rc1[:, :], in_=rd1[:, :])
                b_ps = ps.tile([128, RPC], F32, tag="b", name="b_ps")
                nc.tensor.matmul(b_ps[0:64, :], lhsT=ones_sb[:, :], rhs=rc0[:, :],
                                 start=True, stop=True, tile_position=(0, 0))
                nc.tensor.matmul(b_ps[64:128, :], lhsT=ones_sb[:, :], rhs=rc1[:, :],
                                 start=True, stop=True, tile_position=(0, 64))
                b_sb = sb.tile([128, RPC], F32, tag="bsb", bufs=2, name="b_sb")
                nc.vector.tensor_copy(b_sb[:, :], b_ps[:, :])
                nc.vector.tensor_tensor(
                    attn_sb[0:64, ts(p_i, RPC)], av0[0:64, :], b_sb[0:64, :], MULT)
                nc.vector.tensor_tensor(
                    attn_sb[64:128, ts(p_i, RPC)], av1[0:64, :], b_sb[64:128, :],
                    MULT)

            # ---- output projection: out[rows, H] = attn^T.T @ w_out ----
            for rc_i in range(4):
                o_ps = ps.tile([128, H], F32, tag="s", name="o_ps")
                for nh in range(2):
                    for hc in range(8):
                        nc.tensor.matmul(
                            o_ps[:, ts(nh, 512)],
                            lhsT=attn_sb[:, hc * RPC + rc_i * 128:][:, :128],
                            rhs=wout_sb[:, hc * H + nh * 512:][:, :512],
                            start=(hc == 0),
                            stop=(hc == 7),
                        )
                o_sb = sb.tile([128, H], F32, tag="osb", bufs=2, name="o_sb")
                nc.vector.tensor_copy(o_sb[:, :], o_ps[:, :])
                nc.sync.dma_start(out[ts(rc_i, 128), :], o_sb[:, :])

    nc.finalize()
    return nc


def _av_mm(nc, vt_all, e, av0, av1, kc, h0, h1):
    nc.tensor.matmul(
        av0[:, :], lhsT=vt_all[:, kc * VW + 65 * h0:][:, :65],
        rhs=e[:, 0:RPC], start=(kc == 0), stop=(kc == KC - 1))
    nc.tensor.matmul(
        av1[:, :], lhsT=vt_all[:, kc * VW + 65 * h1:][:, :65],
        rhs=e[:, RPC : 2 * RPC], start=(kc == 0), stop=(kc == KC - 1))


_NC = None


def _get_nc():
    global _NC
    if _NC is None:
        _NC = build_nc()
    return _NC


def _bf16(a):
    return np.ascontiguousarray(a.astype(ml_dtypes.bfloat16))


def make_in_maps(x, rotary_emb, w_qkv, w_out):
    x = np.asarray(x, np.float32)
    rotary_emb = np.asarray(rotary_emb, np.float32)
    w_qkv = np.asarray(w_qkv, np.float32)
    w_out = np.asarray(w_out, np.float32)
    x_flat = x.reshape(B * N, H)
    wqk_bf = _bf16(w_qkv[:, : 2 * H])
    wv_aug = np.zeros((H, VW), np.float32)
    for h in range(HEADS):
        wv_aug[:, 65 * h : 65 * h + 64] = w_qkv[:, 2 * H + 64 * h : 2 * H + 64 * h + 64]
    wv_bf = _bf16(wv_aug)
    wout_bf = _bf16(w_out)
    cos = np.cos(rotary_emb)  # [N, D]
    sin = np.sin(rotary_emb)
    in_maps = []
    for c in range(NC_):
        rows = x_flat[c * RPC : (c + 1) * RPC]
        n0 = (c % 4) * RPC
        cosT = cos[n0 : n0 + RPC].T.astype(np.float32)  # [64, 512]
        sinT = sin[n0 : n0 + RPC].T.astype(np.float32)
        cos2_a = np.ascontiguousarray(np.concatenate([cosT, cosT], axis=0))
        # sswp[p]: p%64<32 -> +sin[p%64+32]; else -> -sin[p%64-32]
        sswp = np.concatenate([sinT[32:], -sinT[:32]], axis=0)
        sinm_a = np.ascontiguousarray(np.concatenate([sswp, sswp], axis=0))
        in_maps.append({
            "xT": _bf16(rows.T),
            "wqk": wqk_bf,
            "wv": wv_bf,
            "wout": wout_bf,
            "cos2": cos2_a,
            "sinm": sinm_a,
        })
    return in_maps


def run(x, rotary_emb, w_qkv, w_out, trace=False, tmpdir=None):
    nc = _get_nc()
    in_maps = make_in_maps(x, rotary_emb, w_qkv, w_out)
    res = run_bass_kernel_spmd(nc, in_maps, list(range(NC_)), trace=trace,
                               tmpdir=tmpdir)
    outs = [np.asarray(res.results[c]["out"], np.float32) for c in range(NC_)]
    full = np.concatenate(outs, axis=0).reshape(B, N, H)
    return full, res


def kernel(x, rotary_emb, w_qkv, w_out):
    full, _ = run(x, rotary_emb, w_qkv, w_out)
    return full
